# revision 7
# baseline (speedup 1.0000x reference)
"""Multi-head attention (softmax over the query axis) on 8 trn2 cores.

Sharding: tensor-parallel over heads — 2 heads per core. Each core computes
its heads' projections + attention + a partial output projection (row-parallel
Wo); the host sums the 8 partial outputs and adds bo.

Device-side layout choices (host pre-packs everything):
  - activations are shipped TRANSPOSED (d on partitions) as bf16, so every
    matmul contracts over the partition dim with natural-layout DMAs.
  - scores are computed transposed ([t, s]) so the softmax axis (query s) is
    the free axis.
  - 1/rowsum is folded into V's rows (16K elems) instead of the attention
    matrix (4.2M elems).

Schedule (v2):
  - all weights packed dt-major and streamed in chunks interleaved with the
    activation strips, so the first projection matmul starts as early as
    possible and no weight load sits on the P-Q/P-K critical DMA window.
  - psum-bank drains (bias adds) moved off the Scalar engine onto
    GpSimd/Vector; Scalar does exp only.
  - exp row-sums moved off the ACT accumulator onto GpSimd tensor_reduce.
  - phase B fuses scores-h1 + pass2-h0 (c-major over resident h0 exps) +
    pass2-h1 (i-major, one-strip lag, 3-deep exp ring) in exactly 8 psum
    banks, absorbing the old serial S2 phases.
  - O phase: h-outer stationary reuse, double-buffered [128,1024] psum
    tiles per output half, output DMA streams during the matmuls.
"""

import json

import numpy as np
import ml_dtypes

import concourse.bass as bass
import concourse.mybir as mybir
import concourse.tile as tile
from concourse import bass_utils

BF16 = mybir.dt.bfloat16
F32 = mybir.dt.float32
AF = mybir.ActivationFunctionType
ALU = mybir.AluOpType
AX = mybir.AxisListType

N_CORES = 8
H = 16
D = 2048
DK = 128
S = 2048
HPC = H // N_CORES          # heads per core = 2
NT = D // 128               # 16 tiles along d / t
NSC = S // 512              # 4 chunks of 512 along s / m
SCALE = 1.0 / float(np.sqrt(DK))

TRACE = False
LAST_RESULTS = None
PHASE_MARKS = []


def _mark(nc, label):
    PHASE_MARKS.append((label, nc.next_id()))


# The walrus in this container accepts only ONE sem-wait per instruction
# (setupSyncWait: "Too many sync wait commands"), but Tile attaches one wait
# per depended-on semaphore. Split extra waits onto single-wait NoOps inserted
# just before the instruction on the same engine, at BIR-JSON level so every
# compile path (native + bass2jax/axon) is covered.
def _split_multi_waits(raw: bytes) -> bytes:
    m = json.loads(raw)
    ctr = 0
    changed = False
    for fn in m.get("functions", []):
        for blk in fn.get("blocks", []):
            insts = blk.get("instructions", [])
            out = []
            for inst in insts:
                si = inst.get("sync_info")
                waits = (si.get("on_wait") or []) if si else []
                if len(waits) > 1:
                    changed = True
                    for w in waits[:-1]:
                        ctr += 1
                        out.append(
                            {
                                "debug": inst.get("debug"),
                                "engine": inst["engine"],
                                "ins": [],
                                "name": f"I-wsplit-{ctr}",
                                "opcode": "NoOp",
                                "outs": [],
                                "sync_info": {"on_update": [], "on_wait": [w]},
                            }
                        )
                    si["on_wait"] = [waits[-1]]
                out.append(inst)
            if changed:
                blk["instructions"] = out
    if not changed:
        return raw
    return json.dumps(m).encode()


_orig_to_json_bytes = bass.Bass.to_json_bytes


def _to_json_bytes_split(self):
    return _split_multi_waits(_orig_to_json_bytes(self))


bass.Bass.to_json_bytes = _to_json_bytes_split


def _build_bass():
    nc = bass.Bass(trn_type="TRN2")

    qT = nc.dram_tensor("qT", [D, S], BF16, kind="ExternalInput")
    kT = nc.dram_tensor("kT", [D, S], BF16, kind="ExternalInput")
    vT = nc.dram_tensor("vT", [D, S], BF16, kind="ExternalInput")
    # all three projection weight stacks packed dt-major:
    # col = (dt*HPC + h)*128 + k, row = d within the dt strip
    wq = nc.dram_tensor("wq", [128, NT * HPC * 128], BF16, kind="ExternalInput")
    wk = nc.dram_tensor("wk", [128, NT * HPC * 128], BF16, kind="ExternalInput")
    wv2 = nc.dram_tensor("wv2", [128, NT * HPC * 128], BF16, kind="ExternalInput")
    wo = nc.dram_tensor("wo", [128, HPC * D], BF16, kind="ExternalInput")
    bqk = nc.dram_tensor("bqk", [128, 2 * HPC], F32, kind="ExternalInput")
    bvb = nc.dram_tensor("bvb", [128, HPC * 128], F32, kind="ExternalInput")
    out = nc.dram_tensor("out_p", [S, D], BF16, kind="ExternalOutput")

    WCH = NT * HPC * 128 // 4  # weight chunk: 4 dt strips = [128, 1024]

    with tile.TileContext(nc) as tc:
        with (
            tc.tile_pool(name="wpool", bufs=1) as wpool,
            tc.tile_pool(name="acts", bufs=1) as acts,
            tc.tile_pool(name="xpool", bufs=4) as xpool,
            tc.tile_pool(name="small", bufs=2) as small,
            tc.tile_pool(name="opool", bufs=2) as opool,
            tc.tile_pool(name="exppool", bufs=1) as exppool,
        ):
            # --- resident weights ---
            wq_sb = wpool.tile([128, NT * HPC * 128], BF16)
            wk_sb = wpool.tile([128, NT * HPC * 128], BF16)
            wv2_sb = wpool.tile([128, NT * HPC * 128], BF16)
            wo_sb = wpool.tile([128, HPC * D], BF16)
            bqk_sb = wpool.tile([128, 2 * HPC], F32)
            bvb_sb = wpool.tile([128, HPC * 128], F32)

            # --- resident per-head activations ---
            QT = [acts.tile([128, S], BF16, name=f"QT{h}") for h in range(HPC)]
            KT = [acts.tile([128, S], BF16, name=f"KT{h}") for h in range(HPC)]
            V = [acts.tile([128, NT * 128], BF16, name=f"V{h}") for h in range(HPC)]
            HT = [acts.tile([128, S], BF16, name=f"HT{h}") for h in range(HPC)]
            vsca = [
                acts.tile([128, NT * 128], BF16, name=f"vsca{h}") for h in range(HPC)
            ]
            # h0's exp strips persist through phase B; h1 uses a 3-deep ring
            EXP0 = [
                exppool.tile([128, S], BF16, name=f"exp0_{i}", tag=f"exp0_{i}", bufs=1)
                for i in range(NT)
            ]

            # ---------------- phases P-Q / P-K: Q^T / K^T projections ----------
            # Per-head psum pools (4 banks each) so phase-A pools map onto
            # per-head release zones. DMA emission order is the schedule:
            # xs strips interleaved with weight chunks.
            with (
                tc.tile_pool(name="pq0", bufs=1, space="PSUM") as pq0,
                tc.tile_pool(name="pq1", bufs=1, space="PSUM") as pq1,
            ):
                pqp = [pq0, pq1]
                for xdram, w_sb, dst, bcol, label in (
                    (qT, wq_sb, QT, 0, "P-Q"),
                    (kT, wk_sb, KT, HPC, "P-K"),
                ):
                    _mark(nc, label)
                    is_q = xdram is qT
                    ps = [
                        [
                            pqp[h].tile(
                                [128, 512], F32, name=f"pp{h}{c}", tag=f"pp{h}{c}",
                                bufs=1,
                            )
                            for c in range(NSC)
                        ]
                        for h in range(HPC)
                    ]
                    for dt in range(NT):
                        xs = xpool.tile([128, S], BF16, name="xs", tag="xs", bufs=6)
                        nc.sync.dma_start(xs[:], xdram[dt * 128 : (dt + 1) * 128, :])
                        if is_q:
                            # interleave wq chunks 0-3 then wk chunks 0-3 into
                            # the q-strip stream; tiny bqk rides along late
                            if dt < 4:
                                nc.sync.dma_start(
                                    wq_sb[:, dt * WCH : (dt + 1) * WCH],
                                    wq[:, dt * WCH : (dt + 1) * WCH],
                                )
                            elif dt < 8:
                                j = dt - 4
                                nc.sync.dma_start(
                                    wk_sb[:, j * WCH : (j + 1) * WCH],
                                    wk[:, j * WCH : (j + 1) * WCH],
                                )
                            elif dt == 8:
                                nc.sync.dma_start(bqk_sb[:], bqk[:])
                        else:
                            # wv2 chunks ride the late k-strip stream
                            if dt >= 12:
                                j = dt - 12
                                nc.sync.dma_start(
                                    wv2_sb[:, j * WCH : (j + 1) * WCH],
                                    wv2[:, j * WCH : (j + 1) * WCH],
                                )
                        for h in range(HPC):
                            for c in range(NSC):
                                nc.tensor.matmul(
                                    ps[h][c][:],
                                    w_sb[:, (dt * HPC + h) * 128 : (dt * HPC + h + 1) * 128],
                                    xs[:, c * 512 : (c + 1) * 512],
                                    start=(dt == 0),
                                    stop=(dt == NT - 1),
                                )
                    # drains: h0 on vector (frees pq0 zone first for phase-A
                    # scores), h1 on scalar; gpsimd cannot read PSUM.
                    for h in range(HPC):
                        for c in range(NSC):
                            if h == 0:
                                nc.vector.tensor_scalar_add(
                                    dst[h][:, c * 512 : (c + 1) * 512],
                                    ps[h][c][:],
                                    bqk_sb[:, bcol + h : bcol + h + 1],
                                )
                            else:
                                nc.scalar.activation(
                                    dst[h][:, c * 512 : (c + 1) * 512],
                                    ps[h][c][:],
                                    AF.Identity,
                                    bias=bqk_sb[:, bcol + h : bcol + h + 1],
                                    scale=1.0,
                                )

            # ---------------- phase A: V projection + scores/exp h0 ----------
            # pscA allocated first -> lands on pq0's (h0) zone; psv second ->
            # pq1's (h1) zone.
            with (
                tc.tile_pool(name="pscA", bufs=1, space="PSUM") as pscA,
                tc.tile_pool(name="ppsv", bufs=1, space="PSUM") as ppsv,
            ):
                _mark(nc, "A")
                nc.sync.dma_start(bvb_sb[:], bvb[:])
                nc.sync.dma_start(wo_sb[:], wo[:])

                rects = {}

                def emit_scores(h, i, expt):
                    # scores strip i -> exp; row-sum as one full-strip DVE
                    # reduce (Scalar does exp only)
                    psc_pool = pscA if h == 0 else pscB
                    for half in range(2):
                        if h == 0:
                            psc = psc_pool.tile(
                                [128, 1024], F32, name=f"psc{half}", tag=f"psc{half}",
                                bufs=1,
                            )
                        else:
                            psc = psc_pool.tile(
                                [128, 1024], F32, name="pscB", tag="pscB", bufs=1
                            )
                        for cc in range(2):
                            c = half * 2 + cc
                            nc.tensor.matmul(
                                psc[:, cc * 512 : (cc + 1) * 512],
                                KT[h][:, i * 128 : (i + 1) * 128],
                                QT[h][:, c * 512 : (c + 1) * 512],
                                start=True,
                                stop=True,
                            )
                        nc.scalar.activation(
                            expt[:, half * 1024 : (half + 1) * 1024],
                            psc[:],
                            AF.Exp,
                            scale=SCALE,
                        )
                    rect = small.tile(
                        [128, 1], F32, name="rect", tag=f"rec{i % 4}", bufs=2
                    )
                    rects[(h, i)] = rect
                    nc.vector.reduce_sum(rect[:], expt[:], axis=AX.X)
                    nc.vector.reciprocal(rect[:], rect[:])

                def emit_vscale(h, i):
                    nc.vector.tensor_scalar_mul(
                        vsca[h][:, i * 128 : (i + 1) * 128],
                        V[h][:, i * 128 : (i + 1) * 128],
                        rects[(h, i)][:],
                    )

                def emit_v_group(tg):
                    # V: [t, hk] natural layout, both heads fused per matmul
                    psv = [
                        ppsv.tile(
                            [128, 512], F32, name=f"psv{tt}", tag=f"psv{tt}", bufs=1
                        )
                        for tt in range(4)
                    ]
                    for dt in range(NT):
                        xc = xpool.tile([128, 512], BF16, name="xc", tag="xc", bufs=12)
                        nc.sync.dma_start(
                            xc[:], vT[dt * 128 : (dt + 1) * 128, tg * 512 : (tg + 1) * 512]
                        )
                        for tt in range(4):
                            nc.tensor.matmul(
                                psv[tt][:, : HPC * 128],
                                xc[:, tt * 128 : (tt + 1) * 128],
                                wv2_sb[:, dt * HPC * 128 : (dt + 1) * HPC * 128],
                                start=(dt == 0),
                                stop=(dt == NT - 1),
                            )
                    for tt in range(4):
                        t_tile = tg * 4 + tt
                        for h in range(HPC):
                            nc.vector.tensor_tensor(
                                V[h][:, t_tile * 128 : (t_tile + 1) * 128],
                                psv[tt][:, h * 128 : (h + 1) * 128],
                                bvb_sb[:, h * 128 : (h + 1) * 128],
                                op=ALU.add,
                            )

                for g in range(4):
                    for i in range(4 * g, 4 * g + 4):
                        emit_scores(0, i, EXP0[i])
                    emit_v_group(g)
                    for i in range(4 * g, 4 * g + 4):
                        emit_vscale(0, i)

            # ---------------- phase B: scores/exp h1 + pass2 h0 + pass2 h1 ---
            # Banks: pscB [128,1024] (2) + ph1 4x[128,512] (4) + ph0
            # [128,512] bufs=2 (2) = 8.
            with (
                tc.tile_pool(name="ppscB", bufs=1, space="PSUM") as pscB,
                tc.tile_pool(name="pph1", bufs=1, space="PSUM") as pph1,
                tc.tile_pool(name="pph0", bufs=1, space="PSUM") as pph0,
            ):
                _mark(nc, "B")
                ph1 = [
                    pph1.tile([128, 512], F32, name=f"ph1{c}", tag=f"ph1{c}", bufs=1)
                    for c in range(NSC)
                ]
                eh1 = {}
                ph0_tile = None

                def emit_pass2_h1(i):
                    nonlocal_expt = eh1.pop(i)
                    for c in range(NSC):
                        nc.tensor.matmul(
                            ph1[c][:],
                            vsca[1][:, i * 128 : (i + 1) * 128],
                            nonlocal_expt[:, c * 512 : (c + 1) * 512],
                            start=(i == 0),
                            stop=(i == NT - 1),
                        )

                for i in range(NT):
                    # scores + exp for h1 strip i (3-deep exp ring)
                    expt = exppool.tile([128, S], BF16, name=f"eh1_{i}", tag="eh1", bufs=3)
                    eh1[i] = expt
                    emit_scores(1, i, expt)
                    emit_vscale(1, i)
                    # pass2-h0, c-major: chunk c = i//4, strips j in 4 per step
                    c = i // 4
                    if i % 4 == 0:
                        ph0_tile = pph0.tile(
                            [128, 512], F32, name=f"ph0{c}", tag="ph0", bufs=2
                        )
                    for j in range(4 * (i % 4), 4 * (i % 4) + 4):
                        nc.tensor.matmul(
                            ph0_tile[:],
                            vsca[0][:, j * 128 : (j + 1) * 128],
                            EXP0[j][:, c * 512 : (c + 1) * 512],
                            start=(j == 0),
                            stop=(j == NT - 1),
                        )
                    if i % 4 == 3:
                        if c % 2 == 0:
                            nc.vector.tensor_copy(
                                HT[0][:, c * 512 : (c + 1) * 512], ph0_tile[:]
                            )
                        else:
                            nc.scalar.copy(
                                HT[0][:, c * 512 : (c + 1) * 512], ph0_tile[:]
                            )
                    # pass2-h1 with one-strip lag
                    if i > 0:
                        emit_pass2_h1(i - 1)
                emit_pass2_h1(NT - 1)
                for c in range(NSC):
                    if c % 2 == 0:
                        nc.vector.tensor_copy(
                            HT[1][:, c * 512 : (c + 1) * 512], ph1[c][:]
                        )
                    else:
                        nc.scalar.copy(HT[1][:, c * 512 : (c + 1) * 512], ph1[c][:])

            # ---------------- phase O: partial output projection --------------
            with tc.tile_pool(name="ppo", bufs=1, space="PSUM") as ppo:
                _mark(nc, "O")
                for st in range(NT):
                    po = [
                        ppo.tile([128, 1024], F32, name=f"po{cp}", tag=f"po{cp}", bufs=2)
                        for cp in range(2)
                    ]
                    for h in range(HPC):
                        for cp in range(2):
                            for cc in range(2):
                                nc.tensor.matmul(
                                    po[cp][:, cc * 512 : (cc + 1) * 512],
                                    HT[h][:, st * 128 : (st + 1) * 128],
                                    wo_sb[
                                        :,
                                        h * D + cp * 1024 + cc * 512 : h * D
                                        + cp * 1024
                                        + (cc + 1) * 512,
                                    ],
                                    start=(h == 0),
                                    stop=(h == HPC - 1),
                                )
                    for cp in range(2):
                        ot = opool.tile([128, 1024], BF16, name="ot", tag="ot", bufs=4)
                        if cp % 2 == 0:
                            nc.vector.tensor_copy(ot[:], po[cp][:])
                        else:
                            nc.scalar.copy(ot[:], po[cp][:])
                        nc.sync.dma_start(
                            out[st * 128 : (st + 1) * 128, cp * 1024 : (cp + 1) * 1024],
                            ot[:],
                        )

    return nc


_NC = None


def _get_nc():
    global _NC
    if _NC is None:
        _NC = _build_bass()
    return _NC


def _prep_inputs(query, key, value, Wq, bq, Wk, bk, Wv, bv, Wo, bo):
    """Host-side shard + pack. Returns per-core input maps."""
    bf = ml_dtypes.bfloat16
    f32 = np.float32

    query = np.asarray(query, f32)
    key = np.asarray(key, f32)
    value = np.asarray(value, f32)
    Wq = np.asarray(Wq, f32)
    Wk = np.asarray(Wk, f32)
    Wv = np.asarray(Wv, f32)
    Wo = np.asarray(Wo, f32)
    bq = np.asarray(bq, f32)
    bk = np.asarray(bk, f32)
    bv = np.asarray(bv, f32)

    qT = np.ascontiguousarray(query.T).astype(bf)
    kT = np.ascontiguousarray(key.T).astype(bf)
    vT = np.ascontiguousarray(value.T).astype(bf)

    in_maps = []
    for c in range(N_CORES):
        heads = [c * HPC + j for j in range(HPC)]

        # dt-major packing for all three stacks:
        # col = (dt*HPC + h)*128 + k, row = d within tile
        def pack_w(W):
            return np.concatenate(
                [
                    np.concatenate(
                        [W[hh].reshape(NT, 128, DK)[dt] for hh in heads], axis=1
                    )
                    for dt in range(NT)
                ],
                axis=1,
            ).astype(bf)

        wo_p = np.concatenate(
            [Wo[hh * DK : (hh + 1) * DK, :] for hh in heads], axis=1
        ).astype(bf)

        bqk = np.stack(
            [bq[hh] for hh in heads] + [bk[hh] for hh in heads], axis=1
        ).astype(f32)
        bvb = np.concatenate(
            [np.broadcast_to(bv[hh][None, :], (128, DK)) for hh in heads], axis=1
        ).astype(f32)

        in_maps.append(
            {
                "qT": qT,
                "kT": kT,
                "vT": vT,
                "wq": np.ascontiguousarray(pack_w(Wq)),
                "wk": np.ascontiguousarray(pack_w(Wk)),
                "wv2": np.ascontiguousarray(pack_w(Wv)),
                "wo": np.ascontiguousarray(wo_p),
                "bqk": np.ascontiguousarray(bqk),
                "bvb": np.ascontiguousarray(bvb),
            }
        )
    return in_maps


def kernel(query, key, value, Wq, bq, Wk, bk, Wv, bv, Wo, bo):
    global LAST_RESULTS
    in_maps = _prep_inputs(query, key, value, Wq, bq, Wk, bk, Wv, bv, Wo, bo)
    nc = _get_nc()
    res = bass_utils.run_bass_kernel_spmd(
        nc, in_maps, core_ids=list(range(N_CORES)), trace=TRACE
    )
    LAST_RESULTS = res
    acc = res.results[0]["out_p"].astype(np.float32)
    for c in range(1, N_CORES):
        acc += res.results[c]["out_p"].astype(np.float32)
    acc += np.asarray(bo, np.float32)[None, :]
    return acc
